# revision 1
# baseline (speedup 1.0000x reference)
"""Trainium2 Bass kernel for nn_DualWeightAttention (B=2, S=2048, H=2048, 16 heads).

Sharding: tensor-parallel over heads — 2 heads per core on 8 cores.
Each core computes q/k/v projections for its 2 heads, attention for those
heads (both batches), and a partial output projection against its 256-row
slice of Wo.T. The 8 partial [4096, 2048] f32 outputs are summed on the host.

On-chip layouts (per core), default dtype fp16 (same PE rate as bf16,
8x finer mantissa; every tensor here is O(1)-scaled so range is ample):
  qT, kT  [128(d), head, B*S]  fp16  (head dim on partitions)
  v       [128(s), tile, 256]  fp16  (seq on partitions)
  scoresT [128(k), q]          psum f32 = (kT k-tile).T @ qT chunk
  attn_u  [128(k), kt, 512]    fp16  = exp(scoresT) * exp(maskT)   (host
                                       precomputes exp(mask); ScalarE's exp
                                       evacuates PSUM, DVE multiplies fp16
                                       SBUF*SBUF in its 2x mode)
  uT      [128(d), head, S]    fp16  = ((attn_u @ v)^T) * 1/denom
  out     [128(s), 512]        f32   = uT.T @ WoT-slice (2-head accumulate)

Softmax is unnormalized: the denominator comes from a ones-vector matmul
over attn_u accumulated across k-tiles, 1/denom via a DVE approx
reciprocal, broadcast across partitions on GpSimd, and applied while
evacuating the PV accumulator.

The phase-2 emission is software-pipelined: period i interleaves QK(i+1)
matmuls with PV(i) matmuls, then the denominator block, then the
out-projection rows of a finished q-chunk — keeping TensorE ~90% busy.
"""

import numpy as np

import concourse.mybir as mybir
import concourse.tile as tile
from concourse import bacc
from concourse.bass_utils import run_bass_kernel_spmd

P = 128
B = 2
S = 2048
H = 2048
NH = 16
HD = 128
NCORES = 8
HPC = NH // NCORES  # heads per core
DC = HPC * HD       # d-columns per core
QC = 512            # q-chunk (matmul moving free dim)
HT = H // P         # contraction tiles for projections
SCALE = 1.0 / float(np.sqrt(HD))

F32 = mybir.dt.float32
F32R = mybir.dt.float32r
# fp16 over bf16: same PE/DVE rates, 8x finer mantissa; all tensors here are
# O(1)-scaled so fp16's range is ample
BF16 = mybir.dt.float16

# matmul operand dtypes (bf16 streams at the same rate as f32r but its
# weight loads overlap cleanly and DMA/SBUF footprints halve)
import os as _os
_DT_MODE = _os.environ.get("ATTN_DT", "bf16")
PROJ_DT = F32R if _DT_MODE in ("f32r", "mixed") else BF16  # hsT + wq/wk/wv
QK_DT = F32R if _DT_MODE in ("f32r", "mixed") else BF16    # qT/kT operands
OUT_DT = F32R if _DT_MODE == "f32r" else BF16              # uT + woT
MASK_DT = F32 if _os.environ.get("ATTN_MASK_DT", "f16") == "f32" else BF16
EXP = mybir.ActivationFunctionType.Exp
ADD = mybir.AluOpType.add
MULT = mybir.AluOpType.mult


def build_attention_nc(s=S):
    bs = B * s
    kt_n = s // P   # k tiles per batch
    nq = s // QC    # q chunks per batch
    st_n = s // P   # s tiles per batch (out projection)
    vt_n = bs // P  # v tiles (both batches)

    nc = bacc.Bacc("TRN2", target_bir_lowering=False, debug=False, num_devices=NCORES)
    hsT = nc.dram_tensor("hsT", [H, bs], PROJ_DT, kind="ExternalInput")
    maskT = nc.dram_tensor("maskT", [B, s, s], MASK_DT, kind="ExternalInput")
    wqT = nc.dram_tensor("wqT", [H, DC], PROJ_DT, kind="ExternalInput")
    wkT = nc.dram_tensor("wkT", [H, DC], PROJ_DT, kind="ExternalInput")
    wvT = nc.dram_tensor("wvT", [H, DC], PROJ_DT, kind="ExternalInput")
    woT = nc.dram_tensor("woT", [DC, H], OUT_DT, kind="ExternalInput")
    out = nc.dram_tensor("out", [bs, H], F32, kind="ExternalOutput")

    hsT_r = hsT.ap().rearrange("(o p) t -> p o t", p=P)
    wq_r = wqT.ap().rearrange("(o p) d -> p o d", p=P)
    wk_r = wkT.ap().rearrange("(o p) d -> p o d", p=P)
    wv_r = wvT.ap().rearrange("(o p) d -> p o d", p=P)
    wo_r = woT.ap().rearrange("(h p) j -> p h j", p=P)
    out_r = out.ap().rearrange("(t p) j -> p t j", p=P)

    with tile.TileContext(nc) as tc:
        with (
            tc.tile_pool(name="const", bufs=1) as constp,
            tc.tile_pool(name="persist", bufs=1) as persist,
        ):
            ones_bf = constp.tile([P, 1], BF16)
            nc.vector.memset(ones_bf[:], 1.0)

            qT = persist.tile([P, HPC, bs], QK_DT)
            kT = persist.tile([P, HPC, bs], QK_DT)
            vsb = persist.tile([P, vt_n, DC], BF16)
            wo_sb = persist.tile([P, HPC, H], OUT_DT)

            # evacuation helper: alternate DVE/ACT so neither paces the PE
            def evac(idx, dst, src):
                if idx % 2 == 0:
                    nc.scalar.copy(dst, src)
                else:
                    nc.vector.tensor_copy(dst, src)

            # ---------------- Phase 1: q/k/v projections ----------------
            with (
                tc.tile_pool(name="wpool", bufs=1) as wpool,
                tc.tile_pool(name="hpool", bufs=10) as hpool,
                tc.tile_pool(name="ppsum", bufs=2, space="PSUM") as ppsum,
                tc.tile_pool(name="vpsum", bufs=4, space="PSUM") as vpsum,
            ):
                # DMA order matters at startup: the first q-projection group
                # only needs wq + the first hsT quarter, so issue those first
                # and defer wk/wv/wo behind them.
                wq_sb = wpool.tile([P, HT, DC], PROJ_DT, tag="wq")
                wk_sb = wpool.tile([P, HT, DC], PROJ_DT, tag="wk")
                wv_sb = wpool.tile([P, HT, DC], PROJ_DT, tag="wv")
                # split the wq load so the very first matmul group only
                # waits on a quarter of the weights, not the whole tile
                for _wf in range(4):
                    _wsl = slice(_wf * (HT // 4), (_wf + 1) * (HT // 4))
                    nc.sync.dma_start(wq_sb[:, _wsl], wq_r[:, _wsl])

                NQT = 4
                KOQ = HT // NQT  # hsT streamed as 4 quarter-K tiles per s-chunk
                for sc in range(bs // QC):
                    ssl = slice(sc * QC, (sc + 1) * QC)
                    quarters = []
                    for qf in range(NQT):
                        hst = hpool.tile([P, KOQ, QC], PROJ_DT, tag="hst")
                        nc.sync.dma_start(
                            hst[:], hsT_r[:, qf * KOQ : (qf + 1) * KOQ, ssl]
                        )
                        quarters.append(hst)
                    if sc == 0:
                        nc.sync.dma_start(wk_sb[:], wk_r)
                        nc.sync.dma_start(wv_sb[:], wv_r)
                        nc.sync.dma_start(wo_sb[:], wo_r)

                    def hq(ko):
                        return quarters[ko // KOQ][:, ko % KOQ]

                    ev = sc  # evac engine round-robin
                    for h in range(HPC):
                        for wsb, dstT in ((wq_sb, qT), (wk_sb, kT)):
                            ps = ppsum.tile([P, QC], F32, tag="psqk")
                            for ko in range(HT):
                                nc.tensor.matmul(
                                    ps[:],
                                    wsb[:, ko, h * P : (h + 1) * P],
                                    hq(ko),
                                    start=(ko == 0),
                                    stop=(ko == HT - 1),
                                )
                            evac(ev, dstT[:, h, ssl], ps[:])
                            ev += 1
                    # v: ko-outer over 4 concurrent PSUM groups so each hsT
                    # quarter is consumed once and can be recycled early
                    psvs = []
                    for st in range(QC // P):
                        psv = vpsum.tile([P, DC], F32, tag="psv")
                        psvs.append(psv)
                    for ko in range(HT):
                        for st in range(QC // P):
                            nc.tensor.matmul(
                                psvs[st][:],
                                hq(ko)[:, st * P : (st + 1) * P],
                                wv_sb[:, ko, :],
                                start=(ko == 0),
                                stop=(ko == HT - 1),
                            )
                    for st in range(QC // P):
                        evac(ev, vsb[:, sc * (QC // P) + st, :], psvs[st][:])
                        ev += 1

            # ---------------- Phase 2: attention + output projection ----------------
            with (
                tc.tile_pool(name="mpool", bufs=6) as mpool,
                tc.tile_pool(name="apool", bufs=3) as apool,
                tc.tile_pool(name="upool", bufs=1) as upool,
                tc.tile_pool(name="rpool", bufs=2) as rpool,
                tc.tile_pool(name="opool", bufs=6) as opool,
                tc.tile_pool(name="spsum", bufs=3, space="PSUM") as spsum,
                tc.tile_pool(name="upsum", bufs=2, space="PSUM") as upsum,
                tc.tile_pool(name="dpsum", bufs=1, space="PSUM") as dpsum,
                tc.tile_pool(name="opsum", bufs=2, space="PSUM") as opsum,
            ):
                units = [
                    (b, qq, h)
                    for b in range(B)
                    for qq in range(nq)
                    for h in range(HPC)
                ]
                nu = len(units)
                mslabs = {}
                aslabs = {}
                psus = {}
                psds = {}
                uTs = {}
                KH = kt_n // 2  # exp emitted per half-slab

                def mask_prefetch(i):
                    b, qq, h = units[i]
                    if h == 0 and (b, qq) not in mslabs:
                        halves = []
                        for mh in range(2):
                            ms = mpool.tile([P, KH, QC], MASK_DT, tag="mslab")
                            nc.sync.dma_start(
                                ms[:],
                                maskT.ap()[b].rearrange("(kt p) q -> p kt q", p=P)[
                                    :, mh * KH : (mh + 1) * KH,
                                    qq * QC : (qq + 1) * QC,
                                ],
                            )
                            halves.append(ms)
                        mslabs[(b, qq)] = halves

                def qk_part(i, kt):
                    # scoresT k-tile matmul; exp(s+m) = exp(s)*exp(m): the
                    # ScalarE exp evacuates PSUM directly and the mask factor
                    # (host-precomputed exp(mask)) is applied as an fp16
                    # SBUF*SBUF multiply, which runs in the DVE's 2x mode --
                    # unlike a PSUM-sourced add, which is locked to 1x
                    b, qq, h = units[i]
                    if kt == 0:
                        asl = apool.tile([P, kt_n, QC], BF16, tag="aslab")
                        aslabs[i] = asl
                    asl = aslabs[i]
                    ms = mslabs[(b, qq)][kt // KH]
                    pss = spsum.tile([P, QC], F32, tag="pss")
                    nc.tensor.matmul(
                        pss[:],
                        kT[:, h, b * s + kt * P : b * s + (kt + 1) * P],
                        qT[:, h, b * s + qq * QC : b * s + (qq + 1) * QC],
                        start=True,
                        stop=True,
                    )
                    nc.scalar.activation(asl[:, kt], pss[:], EXP)
                    nc.vector.tensor_tensor(
                        asl[:, kt], asl[:, kt], ms[:, kt % KH], MULT
                    )

                def pv_part(i, kt):
                    b, qq, h = units[i]
                    asl = aslabs[i]
                    if kt == 0:
                        psu = upsum.tile([P, QC], F32, tag="psu")
                        psus[i] = psu
                    nc.tensor.matmul(
                        psus[i][:],
                        vsb[:, b * kt_n + kt, h * P : (h + 1) * P],
                        asl[:, kt],
                        start=(kt == 0),
                        stop=(kt == kt_n - 1),
                    )

                def den_block(i):
                    # contiguous run keeps the ones-vector loaded across all
                    # 16 matmuls instead of re-loading weights every matmul
                    asl = aslabs[i]
                    psd = dpsum.tile([1, QC], F32, tag="psd")
                    psds[i] = psd
                    for kt in range(kt_n):
                        nc.tensor.matmul(
                            psd[:],
                            ones_bf[:],
                            asl[:, kt],
                            start=(kt == 0),
                            stop=(kt == kt_n - 1),
                        )

                def finish_unit(i):
                    b, qq, h = units[i]
                    aslabs.pop(i)
                    if b not in uTs:
                        uT_new = upool.tile([P, HPC, s], OUT_DT, tag="uT", name="uT")
                        uTs[b] = uT_new
                    # 1/denom: ~51-ULP DVE approx (exact reciprocal is 8
                    # cyc/elem; ScalarE Ln/Exp thrashes ACT table loads),
                    # then replicate across partitions on the idle GpSimd.
                    recip = rpool.tile([1, QC], F32, tag="recip")
                    nc.vector.reciprocal_approx_fast(out=recip[:], in_=psds.pop(i)[:])
                    rbc = rpool.tile([P, QC], F32, tag="rbc")
                    nc.gpsimd.partition_broadcast(rbc[:], recip[:])
                    nc.vector.tensor_tensor(
                        uTs[b][:, h, qq * QC : (qq + 1) * QC],
                        psus.pop(i)[:],
                        rbc[:],
                        MULT,
                    )

                def outproj_chunk(b, qq):
                    # out-projection rows for q-chunk qq only need uT columns
                    # of that chunk, so emit right after its two heads finish
                    # and let it overlap the next chunk's attention periods
                    uT_b = uTs[b]
                    for st in range(qq * (QC // P), (qq + 1) * (QC // P)):
                        for jc in range(H // QC):
                            pso = opsum.tile([P, QC], F32, tag="pso")
                            for h in range(HPC):
                                nc.tensor.matmul(
                                    pso[:],
                                    uT_b[:, h, st * P : (st + 1) * P],
                                    wo_sb[:, h, jc * QC : (jc + 1) * QC],
                                    start=(h == 0),
                                    stop=(h == HPC - 1),
                                )
                            ot = opool.tile([P, QC], F32, tag="ot")
                            evac(st * (H // QC) + jc, ot[:], pso[:])
                            nc.sync.dma_start(
                                out_r[:, b * st_n + st, jc * QC : (jc + 1) * QC], ot[:]
                            )
                    if qq == nq - 1:
                        uTs.pop(b)

                # software pipeline: period i interleaves QK(i+1) with
                # PV/denom(i) at k-tile granularity so the PE fills the
                # DVE-paced QK stalls with ready PV work
                mask_prefetch(0)
                for kt in range(kt_n):
                    qk_part(0, kt)
                for i in range(nu):
                    if i + 1 < nu:
                        mask_prefetch(i + 1)
                    if i + 2 < nu:
                        mask_prefetch(i + 2)
                    for kt in range(kt_n):
                        if i + 1 < nu:
                            qk_part(i + 1, kt)
                        pv_part(i, kt)
                    den_block(i)
                    finish_unit(i)
                    b, qq, h = units[i]
                    if h == HPC - 1:
                        outproj_chunk(b, qq)

    nc.compile()
    return nc


def make_in_maps(hs, mask, Wq, Wk, Wv, Wo):
    """Host-side prep: transpose/shard the full inputs into per-core maps."""
    bs = hs.shape[0] * hs.shape[1]
    proj_np = np.float16 if PROJ_DT == BF16 else np.float32
    out_np = np.float16 if OUT_DT == BF16 else np.float32
    hsT = np.ascontiguousarray(hs.reshape(bs, H).T).astype(proj_np)
    mask_np = np.float32 if MASK_DT == F32 else np.float16
    maskT = np.exp(
        np.ascontiguousarray(mask[:, 0].transpose(0, 2, 1))
    ).astype(mask_np)
    in_maps = []
    for c in range(NCORES):
        sl = slice(c * DC, (c + 1) * DC)
        in_maps.append(
            {
                "hsT": hsT,
                "maskT": maskT,
                "wqT": np.ascontiguousarray((Wq[sl] * SCALE).T).astype(proj_np),
                "wkT": np.ascontiguousarray(Wk[sl].T).astype(proj_np),
                "wvT": np.ascontiguousarray(Wv[sl].T).astype(proj_np),
                "woT": np.ascontiguousarray(Wo[:, sl].T).astype(out_np),
            }
        )
    return in_maps


_NC_CACHE = {}


def get_nc(s=S):
    if s not in _NC_CACHE:
        _NC_CACHE[s] = build_attention_nc(s)
    return _NC_CACHE[s]


def run(hs, mask, Wq, Wk, Wv, Wo, trace=False, trace_kwargs=None):
    s = hs.shape[1]
    nc = get_nc(s)
    in_maps = make_in_maps(hs, mask, Wq, Wk, Wv, Wo)
    res = run_bass_kernel_spmd(
        nc,
        in_maps,
        core_ids=list(range(NCORES)),
        trace=trace,
        **(trace_kwargs or {}),
    )
    parts = np.stack([r["out"] for r in res.results])
    full = parts.sum(axis=0, dtype=np.float64).astype(np.float32)
    return full.reshape(hs.shape[0], s, H), res


def kernel(hidden_states, attention_mask, Wq, Wk, Wv, Wo):
    hs = np.asarray(hidden_states, dtype=np.float32)
    mask = np.asarray(attention_mask, dtype=np.float32)
    Wq = np.asarray(Wq, dtype=np.float32)
    Wk = np.asarray(Wk, dtype=np.float32)
    Wv = np.asarray(Wv, dtype=np.float32)
    Wo = np.asarray(Wo, dtype=np.float32)
    out, _ = run(hs, mask, Wq, Wk, Wv, Wo)
    return out

